# revision 9
# baseline (speedup 1.0000x reference)
"""Trainium2 Bass kernel for CollaborativeAttention (row-sharded, fp8).

Math: with S=512 unique positions and F=T=2048 gathered via fpos/tpos (mod 512),
the whole block collapses to the unique-position problem:
    qf = hs @ Wq ; kf = hs @ Wk ; vf = hs @ Wv + bv ; cbf = hs @ Wcb       [512, *]
    per head h:  w[u, s] = exp(scale*((qf[u]*mix[h]) . kf[s] + cbf[s, h]))
    ctx[u, h*64:(h+1)*64] = (w @ (counts*vf)[:, hcols]) / (w @ counts)
    outfull = ctx @ Wd + bd ; resfull = hs + outfull ; LN  -> normedfull   [512, 1024]
    output  = normedfull[fpos % 512]                                       [2048, 1024]
counts[s] = multiplicity of s in (tpos % 512); the softmax over the 2048 keys
is the count-weighted softmax over the 512 unique keys, and the count weights
are folded multiplicatively into v (numerator) and the z-reduction lhsT
(denominator), so the exp has no per-key bias at all.

Distribution: collectives on this stack are far too slow (~ms), so the kernel
uses a zero-collective row shard: core c owns query rows 64c..64c+63. The k/v
projections (needed in full by every core) are replicated; everything else
(qT, scores, softmax, context, output dense, LayerNorm) is 1/8 per core. The
host concatenates the 8 disjoint row blocks and applies the fpos gather.

Layout/engine choices (cost-model driven — per-instruction fixed overheads on
Act/DVE dominate, so instruction count is minimized):
 - qT is produced directly by PE (lhsT=Wq-block, rhs=hTq), no transposes.
 - mq[d,h,q] = qT*8mix is two big broadcast tensor_tensor ops (one on DVE
   straight from PSUM, one on GpSimd from a small SBUF eviction) against a
   host-precomputed broadcast mixing table; not 128 small DVE ops.
 - the content bias enters the score PSUM via tiny indicator matmuls
   (kTx rows = 8*cb^T, static one-hot rhs), so Exp runs with bias=0 as one
   instruction per PSUM tile (8/iter instead of 64).
 - z and ctx are DoubleRow matmuls into head-aligned PSUM layouts, so the
   softmax normalize is ONE reciprocal + ONE tensor_tensor.
 - 1/std = exp(-0.5*ln(var+eps)): keeps the Act engine on the
   natural_log_exp_and_others table the whole kernel (no per-iter
   ACT_TABLE_LOADs, which cost 1.28us each).

Precision: the residual path keeps the attention output at ~2% of the signal,
so the attention path runs in fp8 e4m3 with DoubleRow matmuls (fp32 PSUM);
the residual + LayerNorm stay fp32. Power-of-2 scale factors keep fp8
operands in the normal range: weights are pre-scaled x128 on the host
(compensated at PSUM eviction), mixing x8 (compensated in the exp scale), the
z-reduction lhsT holds counts/32 so the stored context is x32 (compensated by
a /4096 at the output-dense eviction). bd and bv@Wd are folded into the
host-prepared residual rows; ln_gamma/ln_beta are ones/zeros here (the host
re-applies them if they ever deviate). All weights are SBUF-resident.
"""

import math
import numpy as np

P = 128
S = 512
D = 1024
H = 16
DH = 64
Q = 64               # query rows per core
NB = D // P          # 8 contraction chunks
ND = NB // 2         # 4 DoubleRow double-chunks
NKT = S // P         # 4 key tiles
NPAIR = H // 2       # 8 head pairs
NPP = NPAIR // 2     # 4 pair-pairs (2 pairs share a psum tile)
WCB = D + H          # packed [Wv | Wcb] columns
N_CORES = 8
SCALE = 1.0 / math.sqrt(D / H)  # 0.125
LN_EPS = 1e-5

WSCL = 128.0         # host weight scale (power of 2, keeps fp8 normal-range)
MSCL = 8.0           # host mixing scale
ZSCL = 32.0          # context scale via counts/32 z-reduction
NPOOL = 4            # mq o-blocks computed on GpSimd (rest on DVE)

_CACHE = {}


def _emit(nc, tc, pools, io, it):
    """Emit one full compute iteration (everything after the constant loads)."""
    import concourse.mybir as mybir

    fp = mybir.dt.float32
    f8 = mybir.dt.float8e4
    bf = mybir.dt.bfloat16
    Alu = mybir.AluOpType
    Act = mybir.ActivationFunctionType
    DR = mybir.MatmulPerfMode.DoubleRow

    acts, wp, ps, zp = (pools[k] for k in ("acts", "wp", "ps", "zp"))
    hT = io["hT"]

    # ---- qT projection: qTp[dcol, o, q] = 128*q^T, PE-direct ----
    qTp = ps.tile([P, NB, Q], fp, tag="ps", name=f"qTp{it}")
    for o in range(NB):
        for oo in range(ND):
            nc.tensor.matmul(qTp[:, o, :],
                             lhsT=io["wq"][:, 2 * oo: 2 * oo + 2,
                                           P * o: P * (o + 1)],
                             rhs=io["hTq"][:, 2 * oo: 2 * oo + 2, :],
                             start=(o == 0 and oo == 0),
                             stop=(o == NB - 1 and oo == ND - 1),
                             perf_mode=DR, skip_group_check=True)
    # GpSimd can't read PSUM: evict the Pool-half to SBUF (128*qT, bf16)
    qTs = acts.tile([P, NPOOL, Q], bf, tag="qTs", name=f"qTs{it}")
    nc.vector.tensor_scalar(out=qTs[:], in0=qTp[:, 0:NPOOL, :],
                            scalar1=1.0, scalar2=None, op0=Alu.mult)
    # mq[d, o, h, q] = qT[d,o,q] * 8*mix[d,o,h]: mixb = 8*mix/128 broadcast
    mq = pools["mqp"].tile([P, NB, H, Q], f8, tag="mq", name=f"mq{it}")
    nc.gpsimd.tensor_tensor(
        out=mq[:, 0:NPOOL],
        in0=qTs[:].unsqueeze(2).broadcast_to([P, NPOOL, H, Q]),
        in1=io["mixb"][:, 0:NPOOL], op=Alu.mult)
    nc.vector.tensor_tensor(
        out=mq[:, NPOOL:NB],
        in0=qTp[:, NPOOL:NB, :].unsqueeze(2).broadcast_to(
            [P, NB - NPOOL, H, Q]),
        in1=io["mixb"][:, NPOOL:NB], op=Alu.mult)

    # ---- k projection (full, replicated): kT_sb [d, keys], true scale ----
    kT = acts.tile([P, NB, S], f8, tag="kT", name=f"kT{it}")
    for j in range(NB):
        pk = ps.tile([P, S], fp, tag="ps", name=f"pk{it}_{j}")
        for oo in range(ND):
            nc.tensor.matmul(pk[:],
                             lhsT=io["wk"][:, 2 * oo: 2 * oo + 2,
                                           P * j: P * (j + 1)],
                             rhs=hT[:, 2 * oo: 2 * oo + 2, :],
                             start=(oo == 0), stop=(oo == ND - 1),
                             perf_mode=DR)
        if j % 2 == 0:
            nc.scalar.activation(kT[:, j, :], pk[:], Act.Copy, scale=1.0 / WSCL)
        else:
            nc.vector.tensor_scalar(out=kT[:, j, :], in0=pk[:],
                                    scalar1=1.0 / WSCL, scalar2=None,
                                    op0=Alu.mult)

    # ---- v projection (full, replicated), counts folded in at eviction ----
    v_sb = acts.tile([P, NKT, D], f8, tag="v", name=f"v{it}")
    for kt in range(NKT):
        pv = [ps.tile([P, S], fp, tag="ps", name=f"pv{it}_{kt}_{eh}")
              for eh in range(2)]
        for oo in range(ND):
            lhs = hT[:, 2 * oo: 2 * oo + 2, P * kt: P * (kt + 1)]
            for eh in range(2):
                nc.tensor.matmul(pv[eh][:], lhsT=lhs,
                                 rhs=io["wvcb"][:, 2 * oo: 2 * oo + 2,
                                                S * eh: S * (eh + 1)],
                                 start=(oo == 0), stop=(oo == ND - 1),
                                 perf_mode=DR)
        for eh in range(2):
            # v~ = counts * v  (scale is the per-partition counts/128 vector)
            nc.scalar.activation(v_sb[:, kt, S * eh: S * (eh + 1)], pv[eh][:],
                                 Act.Copy, scale=io["cntW"][:, kt: kt + 1])

    # ---- content bias, transposed: kTx[h, key] = 8*cb[key, h] ----
    pcbT = ps.tile([P, S], fp, tag="ps", name=f"pcbT{it}")
    for oo in range(ND):
        nc.tensor.matmul(pcbT[0:H, :],
                         lhsT=io["wvcb"][:, 2 * oo: 2 * oo + 2, D:WCB],
                         rhs=hT[:, 2 * oo: 2 * oo + 2, :],
                         start=(oo == 0), stop=(oo == ND - 1),
                         perf_mode=DR)
    # rows 16..127 of kTx were zeroed once in the preamble and stay zero
    nc.scalar.activation(io["kTx"][0:H, :], pcbT[0:H, :], Act.Copy,
                         scale=MSCL / WSCL)

    # ---- scores -> exp -> z/ctx, two key-halves ----
    # psum sc[key, pa, ktl, (hh q)] = 8*(q.mix[h].k) + 8*cb[key, h]
    zps = zp.tile([P, NPAIR, P], fp, tag="zp", name=f"zps{it}")
    ctxps = zp.tile([P, NPAIR, P], fp, tag="zp", name=f"ctxps{it}")
    for kh in range(2):
        for pp in range(NPP):
            sc = ps.tile([P, 2, 2, P], fp, tag="ps", name=f"sc{it}_{kh}_{pp}")
            # bias first: one indicator matmul per (pa, ktl) region; the
            # start=True on the first clears the bank's has_written state
            for pa in range(2):
                for ktl in range(2):
                    kt = 2 * kh + ktl
                    pair = 2 * pp + pa
                    nc.tensor.matmul(sc[:, pa, ktl, :],
                                     lhsT=io["kTx"][:, P * kt: P * (kt + 1)],
                                     rhs=io["mqx"][:, pair],
                                     start=(pa == 0 and ktl == 0), stop=False,
                                     skip_group_check=True)
            for oo in range(ND):
                for ktl in range(2):
                    kt = 2 * kh + ktl
                    for pa in range(2):
                        pair = 2 * pp + pa
                        nc.tensor.matmul(
                            sc[:, pa, ktl, :],
                            lhsT=kT[:, 2 * oo: 2 * oo + 2,
                                    P * kt: P * (kt + 1)],
                            rhs=mq[:, 2 * oo: 2 * oo + 2,
                                   2 * pair: 2 * pair + 2, :],
                            start=False,
                            stop=(oo == ND - 1 and ktl == 1 and pa == 1),
                            perf_mode=DR, skip_group_check=True)
            # exp: one instruction for the whole tile (no bias needed)
            wt = wp.tile([P, 2, 2, P], f8, tag="w", name=f"w{it}_{kh}_{pp}")
            nc.scalar.activation(wt[:], sc[:], Act.Exp, scale=SCALE / MSCL)
            # z and ctx accumulate across both key-halves (DoubleRow over
            # the kh's two key tiles); full [128, 128] outs, the diagonal
            # (matching head halves) is extracted in the normalize
            for pa in range(2):
                pair = 2 * pp + pa
                rhs = wt[:, pa, :, :]
                # zps/ctxps each span TWO psum banks (pairs 0-3 / 4-7):
                # the first write into EACH bank must carry start=True
                first = (kh == 0 and pair % 4 == 0)
                last = (kh == 1 and pp == NPP - 1 and pa == 1)
                nc.tensor.matmul(
                    zps[:, pair, :],
                    lhsT=io["cnt8"][:, 2 * kh: 2 * kh + 2, :],
                    rhs=rhs, start=first, stop=last,
                    perf_mode=DR, skip_group_check=True)
                nc.tensor.matmul(
                    ctxps[:, pair, :],
                    lhsT=v_sb[:, 2 * kh: 2 * kh + 2,
                              P * pair: P * (pair + 1)],
                    rhs=rhs, start=first, stop=last,
                    perf_mode=DR, skip_group_check=True)

    # ---- normalize: ctxT[vcol, pair, q] = 32*ctx[vcol, q]/z[head(vcol), q]
    # rz[p, pair, hh, q] = 1/z[head(pair,hh), q] (valid on every partition);
    # the two norm instructions pick the diagonal head half per partition half
    rz = acts.tile([P, NPAIR, 2, DH], fp, tag="rz", name=f"rz{it}")
    nc.vector.reciprocal(rz[:], zps[:])
    ctxT = acts.tile([P, NPAIR, Q], f8, tag="ctxT", name=f"ctxT{it}")
    for hh in range(2):
        r0 = DH * hh
        nc.vector.tensor_tensor(ctxT[r0:r0 + DH, :, :],
                                ctxps[r0:r0 + DH, :, r0:r0 + DH],
                                rz[r0:r0 + DH, :, hh, :], Alu.mult)

    # ---- output projection (psum = 32*ctx_n @ 128*Wd = 4096*out) ----
    po = [ps.tile([Q, S], fp, tag="ps", name=f"po{it}_{eh}")
          for eh in range(2)]
    for oo in range(ND):
        for eh in range(2):
            nc.tensor.matmul(po[eh][:],
                             lhsT=ctxT[:, 2 * oo: 2 * oo + 2, :],
                             rhs=io["wd"][:, 2 * oo: 2 * oo + 2,
                                          S * eh: S * (eh + 1)],
                             start=(oo == 0), stop=(oo == ND - 1),
                             perf_mode=DR)

    # ---- epilogue: residual (hid+bd+bv@Wd host-folded), LayerNorm ----
    r_sb = acts.tile([Q, D], fp, tag="r", name=f"r{it}")
    for eh in range(2):
        nc.vector.scalar_tensor_tensor(
            out=r_sb[:, S * eh: S * (eh + 1)], in0=po[eh][:],
            scalar=1.0 / (WSCL * ZSCL),
            in1=io["hidq_sb"][:, S * eh: S * (eh + 1)],
            op0=Alu.mult, op1=Alu.add)
    stats = acts.tile([Q, 2, 6], fp, tag="stats", name=f"stats{it}")
    nc.vector.bn_stats(stats[:, 0, :], r_sb[:, 0:S])
    nc.vector.bn_stats(stats[:, 1, :], r_sb[:, S:D])
    mv = acts.tile([Q, 2], fp, tag="mv", name=f"mv{it}")
    nc.vector.bn_aggr(mv[:], stats[:])
    # 1/std = exp(-0.5*ln(var+eps)): stays on the exp/ln activation table
    lnv = acts.tile([Q, 1], fp, tag="lnv", name=f"lnv{it}")
    nc.scalar.activation(lnv[:], mv[:, 1:2], Act.Ln,
                         bias=io["eps_t"][0:Q, :], scale=1.0)
    std = acts.tile([Q, 1], fp, tag="std", name=f"std{it}")
    nc.scalar.activation(std[:], lnv[:], Act.Exp, scale=-0.5)
    nc.vector.tensor_scalar(out=r_sb[:], in0=r_sb[:],
                            scalar1=mv[:, 0:1], scalar2=std[:],
                            op0=Alu.subtract, op1=Alu.mult)
    nc.sync.dma_start(io["out"][:], r_sb[:])


def _build(iters=1):
    import concourse.mybir as mybir
    import concourse.tile as tile
    from concourse import bacc

    fp = mybir.dt.float32
    f8 = mybir.dt.float8e4
    bf = mybir.dt.bfloat16

    nc = bacc.Bacc("TRN2", target_bir_lowering=False, debug=False,
                   num_devices=N_CORES)

    hTd = nc.dram_tensor("hT", [D, S], f8, kind="ExternalInput").ap()
    hTqd = nc.dram_tensor("hTq", [D, Q], f8, kind="ExternalInput").ap()
    hidqd = nc.dram_tensor("hidq", [Q, D], fp, kind="ExternalInput").ap()
    wkd = nc.dram_tensor("wk", [D, D], f8, kind="ExternalInput").ap()
    wqd = nc.dram_tensor("wq", [D, D], f8, kind="ExternalInput").ap()
    wvcbd = nc.dram_tensor("wvcb", [D, WCB], f8, kind="ExternalInput").ap()
    wdd = nc.dram_tensor("wd", [D, D], f8, kind="ExternalInput").ap()
    mixbd = nc.dram_tensor("mixb", [P, NB * H * Q], bf,
                           kind="ExternalInput").ap()
    mqxd = nc.dram_tensor("mqx", [P, NPAIR * 2 * Q], f8,
                          kind="ExternalInput").ap()
    cnt8d = nc.dram_tensor("cnt8", [P, NKT * P], f8, kind="ExternalInput").ap()
    cntWd = nc.dram_tensor("cntW", [P, NKT], fp, kind="ExternalInput").ap()
    outd = nc.dram_tensor("out", [Q, D], fp, kind="ExternalOutput").ap()

    with tile.TileContext(nc) as tc:
        with (
            tc.tile_pool(name="singles", bufs=1) as singles,
            tc.tile_pool(name="acts", bufs=2) as acts,
            tc.tile_pool(name="mqp", bufs=2) as mqp,
            tc.tile_pool(name="wp", bufs=8) as wp,
            tc.tile_pool(name="ps", bufs=4, space="PSUM") as ps,
            tc.tile_pool(name="zp", bufs=2, space="PSUM") as zp,
        ):
            pools = {"singles": singles, "acts": acts, "mqp": mqp,
                     "wp": wp, "ps": ps, "zp": zp}
            # ---- constant / input loads (once) ----
            hT = singles.tile([P, NB, S], f8)
            hTr = hTd.rearrange("(o p) u -> p o u", p=P)
            for _o in range(NB):
                nc.sync.dma_start(hT[:, _o, :], hTr[:, _o, :])
            hTq = singles.tile([P, NB, Q], f8)
            nc.sync.dma_start(hTq[:], hTqd.rearrange("(o p) q -> p o q", p=P))
            wk_sb = singles.tile([P, NB, D], f8)
            wq_sb = singles.tile([P, NB, D], f8)
            wd_sb = singles.tile([P, NB, D], f8)
            for wdram, dest in ((wkd, wk_sb), (wqd, wq_sb), (wdd, wd_sb)):
                wr = wdram.rearrange("(o p) m -> p o m", p=P)
                for _o in range(NB):
                    nc.sync.dma_start(dest[:, _o, :], wr[:, _o, :])
            wvcb_sb = singles.tile([P, NB, WCB], f8)
            wvr = wvcbd.rearrange("(o p) m -> p o m", p=P)
            for _o in range(NB):
                nc.sync.dma_start(wvcb_sb[:, _o, :], wvr[:, _o, :])
            mixb_sb = singles.tile([P, NB, H, Q], bf)
            nc.sync.dma_start(mixb_sb[:], mixbd)
            mqx_sb = singles.tile([P, NPAIR, 2, Q], f8)
            nc.sync.dma_start(mqx_sb[:], mqxd)
            cnt8_sb = singles.tile([P, NKT, P], f8)
            nc.sync.dma_start(cnt8_sb[:], cnt8d)
            cntW_sb = singles.tile([P, NKT], fp)
            nc.sync.dma_start(cntW_sb[:], cntWd)
            hidq_sb = singles.tile([Q, D], fp)
            nc.sync.dma_start(hidq_sb[:], hidqd)
            kTx_sb = singles.tile([P, S], f8)
            nc.vector.memset(kTx_sb[:], 0.0)
            eps_t = singles.tile([P, 1], fp)
            nc.vector.memset(eps_t[:], LN_EPS)

            io = {"hT": hT, "hTq": hTq, "wk": wk_sb, "wq": wq_sb,
                  "wvcb": wvcb_sb, "wd": wd_sb, "mixb": mixb_sb,
                  "mqx": mqx_sb, "cnt8": cnt8_sb, "cntW": cntW_sb,
                  "hidq_sb": hidq_sb, "kTx": kTx_sb, "eps_t": eps_t,
                  "out": outd}

            with nc.allow_low_precision(
                    reason="attention path tolerates fp8; residual+LN fp32"):
                for it in range(iters):
                    _emit(nc, tc, pools, io, it)

    nc.compile()
    return nc


def _get_nc(iters=1):
    key = ("nc", iters)
    if key not in _CACHE:
        _CACHE[key] = _build(iters)
    return _CACHE[key]


def _prepare_in_maps(hidden_states, fpos, tpos, Wq, Wk, Wcb, Wv, bv, mixing,
                     Wd, bd, ln_gamma, ln_beta):
    import ml_dtypes
    f8 = ml_dtypes.float8_e4m3
    bf16 = ml_dtypes.bfloat16
    hs = np.ascontiguousarray(np.asarray(hidden_states, dtype=np.float32))
    tidx = np.asarray(tpos).astype(np.int64) % S
    counts = np.bincount(tidx, minlength=S).astype(np.float32)
    wvcb = np.concatenate([np.asarray(Wv, np.float32),
                           np.asarray(Wcb, np.float32)], axis=1)
    # mixb[p, o, h, q] = 8*mix[h, o*128+p]/128, broadcast along q
    mixT = (MSCL / WSCL) * np.asarray(mixing, np.float32).T      # [D, H]
    mixb = np.broadcast_to(
        mixT.reshape(NB, P, H).transpose(1, 0, 2)[:, :, :, None],
        (P, NB, H, Q))
    # mqx[r, pair, hh, q] = 1 if r == 2*pair+hh else 0
    mqx = np.zeros((P, NPAIR, 2, Q), np.float32)
    for pair in range(NPAIR):
        for hh in range(2):
            mqx[2 * pair + hh, pair, hh, :] = 1.0
    # cnt8[p, kt, c] = counts[kt*128+p]/32 ; cntW[p, kt] = counts[kt*128+p]/128
    cgrid = counts.reshape(NKT, P).T                              # [P, NKT]
    cnt8 = np.broadcast_to((cgrid / ZSCL)[:, :, None], (P, NKT, P))
    common = {
        "hT": np.ascontiguousarray(hs.T).astype(f8),
        "wk": (WSCL * np.asarray(Wk, np.float32)).astype(f8),
        "wq": (WSCL * np.asarray(Wq, np.float32)).astype(f8),
        "wvcb": (WSCL * wvcb).astype(f8),
        "wd": (WSCL * np.asarray(Wd, np.float32)).astype(f8),
        "mixb": np.ascontiguousarray(mixb.reshape(P, -1)).astype(bf16),
        "mqx": np.ascontiguousarray(mqx.reshape(P, -1)).astype(f8),
        "cnt8": np.ascontiguousarray(cnt8.reshape(P, -1)).astype(f8),
        "cntW": np.ascontiguousarray(cgrid / WSCL),
    }
    resid_bias = (np.asarray(bd, np.float32)
                  + np.asarray(bv, np.float32)
                  @ np.asarray(Wd, np.float32))
    in_maps = []
    for c in range(N_CORES):
        m = dict(common)
        rows = hs[Q * c: Q * (c + 1)]
        m["hTq"] = np.ascontiguousarray(rows.T).astype(f8)
        m["hidq"] = np.ascontiguousarray(rows + resid_bias[None, :])
        in_maps.append(m)
    return in_maps


def _run(inputs, trace=False, iters=1):
    from concourse import bass_utils
    nc = _get_nc(iters)
    in_maps = _prepare_in_maps(**inputs)
    res = bass_utils.run_bass_kernel_spmd(
        nc, in_maps, core_ids=list(range(N_CORES)), trace=trace)
    normedfull = np.concatenate(
        [np.asarray(res.results[c]["out"]) for c in range(N_CORES)], axis=0)
    gam = np.asarray(inputs["ln_gamma"], np.float32)
    bet = np.asarray(inputs["ln_beta"], np.float32)
    if not (np.all(gam == 1.0) and np.all(bet == 0.0)):
        normedfull = normedfull * gam[None, :] + bet[None, :]
    fidx = np.asarray(inputs["fpos"]).astype(np.int64) % S
    return np.ascontiguousarray(normedfull[fidx]), res


def kernel(**inputs) -> np.ndarray:
    out, _ = _run(inputs, trace=False)
    return out


# revision 18
# speedup vs baseline: 1.1149x; 1.1149x over previous
"""Trainium2 Bass kernel for CollaborativeAttention (row-sharded, fp8).

Math: with S=512 unique positions and F=T=2048 gathered via fpos/tpos (mod 512),
the whole block collapses to the unique-position problem:
    qf = hs @ Wq ; kf = hs @ Wk ; vf = hs @ Wv + bv ; cbf = hs @ Wcb       [512, *]
    per head h:  w[u, s] = exp(scale*((qf[u]*mix[h]) . kf[s] + cbf[s, h]))
    ctx[u, h*64:(h+1)*64] = (w @ (counts*vf)[:, hcols]) / (w @ counts)
    outfull = ctx @ Wd + bd ; resfull = hs + outfull ; LN  -> normedfull   [512, 1024]
    output  = normedfull[fpos % 512]                                       [2048, 1024]
counts[s] = multiplicity of s in (tpos % 512); the softmax over the 2048 keys
is the count-weighted softmax over the 512 unique keys, and the count weights
are folded multiplicatively into v (numerator) and the z-reduction lhsT
(denominator), so the exp has no per-key bias at all.

Distribution: collectives on this stack are far too slow (~ms), so the kernel
uses a zero-collective row shard: core c owns query rows 64c..64c+63. The k/v
projections (needed in full by every core) are replicated; everything else
(qT, scores, softmax, context, output dense, LayerNorm) is 1/8 per core. The
host concatenates the 8 disjoint row blocks and applies the fpos gather.

Layout/engine choices (cost-model driven — per-instruction fixed overheads on
Act/DVE dominate, so instruction count is minimized):
 - qT is produced directly by PE (lhsT=Wq-block, rhs=hTq), no transposes.
 - mq[d,h,q] = qT*8mix is two big broadcast tensor_tensor ops (one on DVE
   straight from PSUM, one on GpSimd from a small SBUF eviction) against a
   host-precomputed broadcast mixing table; not 128 small DVE ops.
 - the content bias enters the score PSUM via tiny indicator matmuls
   (kTx rows = 8*cb^T, static one-hot rhs), so Exp runs with bias=0 as one
   instruction per PSUM tile (8/iter instead of 64).
 - z and ctx are DoubleRow matmuls into head-aligned PSUM layouts, so the
   softmax normalize is ONE reciprocal + ONE tensor_tensor.
 - 1/std = exp(-0.5*ln(var+eps)): keeps the Act engine on the
   natural_log_exp_and_others table the whole kernel (no per-iter
   ACT_TABLE_LOADs, which cost 1.28us each).

Precision: the residual path keeps the attention output at ~2% of the signal,
so the attention path runs in fp8 e4m3 with DoubleRow matmuls (fp32 PSUM);
the residual + LayerNorm stay fp32. Power-of-2 scale factors keep fp8
operands in the normal range: weights are pre-scaled x128 on the host
(compensated at PSUM eviction), mixing x8 (compensated in the exp scale), the
z-reduction lhsT holds counts/32 so the stored context is x32 (compensated by
a /4096 at the output-dense eviction). bd and bv@Wd are folded into the
host-prepared residual rows; ln_gamma/ln_beta are ones/zeros here (the host
re-applies them if they ever deviate). All weights are SBUF-resident.
"""

import math
import numpy as np

P = 128
S = 512
D = 1024
H = 16
DH = 64
Q = 64               # query rows per core
NB = D // P          # 8 contraction chunks
ND = NB // 2         # 4 DoubleRow double-chunks
NKT = S // P         # 4 key tiles
NPAIR = H // 2       # 8 head pairs
NPP = NPAIR // 2     # 4 pair-pairs (2 pairs share a psum tile)
WCB = D + H          # packed [Wv | Wcb] columns
N_CORES = 8
SCALE = 1.0 / math.sqrt(D / H)  # 0.125
LN_EPS = 1e-5

WSCL = 128.0         # host weight scale (power of 2, keeps fp8 normal-range)
MSCL = 8.0           # host mixing scale
ZSCL = 32.0          # context scale via counts/32 z-reduction
NPOOL = 3            # mq o-blocks computed on GpSimd (rest on DVE)

_CACHE = {}


def _emit(nc, tc, pools, io, it):
    """Emit one full compute iteration (everything after the constant loads)."""
    import concourse.mybir as mybir

    fp = mybir.dt.float32
    f8 = mybir.dt.float8e4
    bf = mybir.dt.bfloat16
    Alu = mybir.AluOpType
    Act = mybir.ActivationFunctionType
    DR = mybir.MatmulPerfMode.DoubleRow

    acts, wp, ps, zp = (pools[k] for k in ("acts", "wp", "ps", "zp"))
    hT = io["hT"]

    # ---- qT projection: qTp[dcol, o, q] = 128*q^T, PE-direct ----
    qTp = ps.tile([P, NB, Q], fp, tag="ps", name=f"qTp{it}")
    for o in range(NB):
        for oo in range(ND):
            nc.tensor.matmul(qTp[:, o, :],
                             lhsT=io["wq"][:, 2 * oo: 2 * oo + 2,
                                           P * o: P * (o + 1)],
                             rhs=io["hTq"][:, 2 * oo: 2 * oo + 2, :],
                             start=(o == 0 and oo == 0),
                             stop=(o == NB - 1 and oo == ND - 1),
                             perf_mode=DR, skip_group_check=True)
    # evict the whole qT to SBUF at once (128*qT, bf16) so the psum tile is
    # freed fast (GpSimd can't read PSUM anyway)
    NP0 = NB - NPOOL     # DVE handles blocks [0, NP0), Pool [NP0, NB)
    qTs = acts.tile([P, NB, Q], bf, tag="qTs", name=f"qTs{it}")
    nc.vector.tensor_scalar(out=qTs[:, NP0:NB, :], in0=qTp[:, NP0:NB, :],
                            scalar1=1.0, scalar2=None, op0=Alu.mult)
    nc.vector.tensor_scalar(out=qTs[:, 0:NP0, :], in0=qTp[:, 0:NP0, :],
                            scalar1=1.0, scalar2=None, op0=Alu.mult)
    # mq[d, o, h, q] = qT[d,o,q] * 8*mix[d,o,h]: mixb = 8*mix/128 broadcast
    mq = pools["mqp"].tile([P, NB, H, Q], f8, tag="mq", name=f"mq{it}")
    nc.gpsimd.tensor_tensor(
        out=mq[:, NP0:NB],
        in0=qTs[:, NP0:NB, :].unsqueeze(2).broadcast_to([P, NPOOL, H, Q]),
        in1=io["mixb"][:, NP0:NB], op=Alu.mult)
    nc.vector.tensor_tensor(
        out=mq[:, 0:2],
        in0=qTs[:, 0:2, :].unsqueeze(2).broadcast_to([P, 2, H, Q]),
        in1=io["mixb"][:, 0:2], op=Alu.mult)
    nc.vector.tensor_tensor(
        out=mq[:, 2:NP0],
        in0=qTs[:, 2:NP0, :].unsqueeze(2).broadcast_to([P, NP0 - 2, H, Q]),
        in1=io["mixb"][:, 2:NP0], op=Alu.mult)

    # ---- k projection (full, replicated): kT_sb [d, keys], true scale ----
    kT = acts.tile([P, NB, S], f8, tag="kT", name=f"kT{it}")
    for j in range(NB):
        pk = ps.tile([P, S], fp, tag="ps", name=f"pk{it}_{j}")
        for oo in range(ND):
            nc.tensor.matmul(pk[:],
                             lhsT=io["wk"][:, 2 * oo: 2 * oo + 2,
                                           P * j: P * (j + 1)],
                             rhs=hT[:, 2 * oo: 2 * oo + 2, :],
                             start=(oo == 0), stop=(oo == ND - 1),
                             perf_mode=DR)
        if j % 2 == 0:
            nc.scalar.activation(kT[:, j, :], pk[:], Act.Copy, scale=1.0 / WSCL)
        else:
            nc.vector.tensor_scalar(out=kT[:, j, :], in0=pk[:],
                                    scalar1=1.0 / WSCL, scalar2=None,
                                    op0=Alu.mult)

    # ---- v projection (full, replicated), counts folded in at eviction ----
    v_sb = acts.tile([P, NKT, D], f8, tag="v", name=f"v{it}")
    for kt in range(NKT):
        pv = [ps.tile([P, S], fp, tag="ps", name=f"pv{it}_{kt}_{eh}")
              for eh in range(2)]
        for oo in range(ND):
            lhs = hT[:, 2 * oo: 2 * oo + 2, P * kt: P * (kt + 1)]
            for eh in range(2):
                nc.tensor.matmul(pv[eh][:], lhsT=lhs,
                                 rhs=io["wvcb"][:, 2 * oo: 2 * oo + 2,
                                                S * eh: S * (eh + 1)],
                                 start=(oo == 0), stop=(oo == ND - 1),
                                 perf_mode=DR)
        for eh in range(2):
            # v~ = counts * v  (scale is the per-partition counts/128 vector)
            nc.scalar.activation(v_sb[:, kt, S * eh: S * (eh + 1)], pv[eh][:],
                                 Act.Copy, scale=io["cntW"][:, kt: kt + 1])

    # ---- content bias, transposed: kTx[h, key] = 8*cb[key, h] ----
    pcbT = ps.tile([P, S], fp, tag="ps", name=f"pcbT{it}")
    for oo in range(ND):
        nc.tensor.matmul(pcbT[0:H, :],
                         lhsT=io["wvcb"][:, 2 * oo: 2 * oo + 2, D:WCB],
                         rhs=hT[:, 2 * oo: 2 * oo + 2, :],
                         start=(oo == 0), stop=(oo == ND - 1),
                         perf_mode=DR)
    # rows 16..127 and the whole second DR sub-block of kTx were zeroed once
    # in the preamble and stay zero
    nc.scalar.activation(io["kTx"][0:H, 0, :], pcbT[0:H, :], Act.Copy,
                         scale=MSCL / WSCL)

    # ---- scores -> exp -> z/ctx, two key-halves ----
    # psum sc[key, pa, ktl, (hh q)] = 8*(q.mix[h].k) + 8*cb[key, h]
    zps = zp.tile([P, NPAIR, P], fp, tag="zp", name=f"zps{it}")
    ctxps = zp.tile([P, NPAIR, P], fp, tag="zp", name=f"ctxps{it}")
    for kh in range(2):
        for pp in range(NPP):
            sc = ps.tile([P, 2, 2, P], fp, tag="ps", name=f"sc{it}_{kh}_{pp}")
            # scores first (start=True clears the bank), content-bias DRs
            # last (they gate only the exp, not the score accumulation)
            for oo in range(ND):
                for ktl in range(2):
                    kt = 2 * kh + ktl
                    for pa in range(2):
                        pair = 2 * pp + pa
                        nc.tensor.matmul(
                            sc[:, pa, ktl, :],
                            lhsT=kT[:, 2 * oo: 2 * oo + 2,
                                    P * kt: P * (kt + 1)],
                            rhs=mq[:, 2 * oo: 2 * oo + 2,
                                   2 * pair: 2 * pair + 2, :],
                            start=(oo == 0 and ktl == 0 and pa == 0),
                            stop=False,
                            perf_mode=DR, skip_group_check=True)
            for pa in range(2):
                for ktl in range(2):
                    kt = 2 * kh + ktl
                    pair = 2 * pp + pa
                    nc.tensor.matmul(sc[:, pa, ktl, :],
                                     lhsT=io["kTx"][:, :,
                                                    P * kt: P * (kt + 1)],
                                     rhs=io["mqx"][:, :, pair],
                                     start=False,
                                     stop=(pa == 1 and ktl == 1),
                                     perf_mode=DR, skip_group_check=True)
            # exp: one instruction for the whole tile (no bias needed)
            wt = wp.tile([P, 2, 2, P], f8, tag="w", name=f"w{it}_{kh}_{pp}")
            nc.scalar.activation(wt[:], sc[:], Act.Exp, scale=SCALE / MSCL)
            # z and ctx accumulate across both key-halves (DoubleRow over
            # the kh's two key tiles); full [128, 128] outs, the diagonal
            # (matching head halves) is extracted in the normalize
            for pa in range(2):
                pair = 2 * pp + pa
                rhs = wt[:, pa, :, :]
                # zps/ctxps each span TWO psum banks (pairs 0-3 / 4-7):
                # the first write into EACH bank must carry start=True
                first = (kh == 0 and pair % 4 == 0)
                last = (kh == 1 and pp == NPP - 1 and pa == 1)
                nc.tensor.matmul(
                    zps[:, pair, :],
                    lhsT=io["cnt8"][:, 2 * kh: 2 * kh + 2, :],
                    rhs=rhs, start=first, stop=last,
                    perf_mode=DR, skip_group_check=True)
                nc.tensor.matmul(
                    ctxps[:, pair, :],
                    lhsT=v_sb[:, 2 * kh: 2 * kh + 2,
                              P * pair: P * (pair + 1)],
                    rhs=rhs, start=first, stop=last,
                    perf_mode=DR, skip_group_check=True)

    # ---- normalize: ctxT[vcol, pair, q] = 32*ctx[vcol, q]/z[head(vcol), q]
    # (engines may read only ONE psum operand per instruction, so z goes
    # through a reciprocal into SBUF first; zps holds the same z row on every
    # partition, so each partition half reads its own head half)
    rz = acts.tile([P, NPAIR, P], fp, tag="rz", name=f"rz{it}")
    nc.vector.reciprocal(rz[:], zps[:])
    ctxT = acts.tile([P, NPAIR, Q], f8, tag="ctxT", name=f"ctxT{it}")
    for hh in range(2):
        r0 = DH * hh
        nc.vector.tensor_tensor(ctxT[r0:r0 + DH, :, :],
                                ctxps[r0:r0 + DH, :, r0:r0 + DH],
                                rz[r0:r0 + DH, :, r0:r0 + DH], Alu.mult)

    # ---- output projection (psum = 32*ctx_n @ 128*Wd = 4096*out) ----
    po = [zp.tile([Q, S], fp, tag="zp", name=f"po{it}_{eh}")
          for eh in range(2)]
    for oo in range(ND):
        for eh in range(2):
            nc.tensor.matmul(po[eh][:],
                             lhsT=ctxT[:, 2 * oo: 2 * oo + 2, :],
                             rhs=io["wd"][:, 2 * oo: 2 * oo + 2,
                                          S * eh: S * (eh + 1)],
                             start=(oo == 0), stop=(oo == ND - 1),
                             perf_mode=DR)

    # ---- epilogue: residual (hid+bd+bv@Wd host-folded), LayerNorm ----
    r_sb = acts.tile([Q, D], fp, tag="r", name=f"r{it}")
    for eh in range(2):
        nc.vector.scalar_tensor_tensor(
            out=r_sb[:, S * eh: S * (eh + 1)], in0=po[eh][:],
            scalar=1.0 / (WSCL * ZSCL),
            in1=io["hidq_sb"][:, S * eh: S * (eh + 1)],
            op0=Alu.mult, op1=Alu.add)
    stats = acts.tile([Q, 2, 6], fp, tag="stats", name=f"stats{it}")
    nc.vector.bn_stats(stats[:, 0, :], r_sb[:, 0:S])
    nc.vector.bn_stats(stats[:, 1, :], r_sb[:, S:D])
    mv = acts.tile([Q, 2], fp, tag="mv", name=f"mv{it}")
    nc.vector.bn_aggr(mv[:], stats[:])
    # 1/std via exponent-bits rsqrt + one Newton step (avoids Ln, whose
    # activation table does not contain Exp -> 2x 1.28us table loads/iter):
    #   log2(v) ~ bits(v)/2^23 - 127 ; y0 = exp(-ln2/2 * log2(v))
    #   istd = y0*(1.5 - 0.5*v*y0^2)   (v >> eps here, so eps is dropped)
    y0 = acts.tile([Q, 1], fp, tag="y0", name=f"y0{it}")
    nc.scalar.activation(y0[:], mv[:, 1:2].bitcast(mybir.dt.int32), Act.Exp,
                         scale=-0.5 * math.log(2.0) / (1 << 23),
                         bias=io["rsb_t"][0:Q, :])
    t1 = acts.tile([Q, 1], fp, tag="t1", name=f"t1{it}")
    nc.vector.tensor_tensor(t1[:], y0[:], y0[:], Alu.mult)
    nc.vector.tensor_tensor(t1[:], t1[:], mv[:, 1:2], Alu.mult)
    nc.vector.tensor_scalar(out=t1[:], in0=t1[:], scalar1=-0.5, scalar2=1.5,
                            op0=Alu.mult, op1=Alu.add)
    std = acts.tile([Q, 1], fp, tag="std", name=f"std{it}")
    nc.vector.tensor_tensor(std[:], t1[:], y0[:], Alu.mult)
    nc.vector.tensor_scalar(out=r_sb[:], in0=r_sb[:],
                            scalar1=mv[:, 0:1], scalar2=std[:],
                            op0=Alu.subtract, op1=Alu.mult)
    nc.sync.dma_start(io["out"][:], r_sb[:])


def _build(iters=1):
    import concourse.mybir as mybir
    import concourse.tile as tile
    from concourse import bacc

    fp = mybir.dt.float32
    f8 = mybir.dt.float8e4
    bf = mybir.dt.bfloat16

    nc = bacc.Bacc("TRN2", target_bir_lowering=False, debug=False,
                   num_devices=N_CORES)

    hTd = nc.dram_tensor("hT", [D, S], f8, kind="ExternalInput").ap()
    hTqd = nc.dram_tensor("hTq", [D, Q], f8, kind="ExternalInput").ap()
    hidqd = nc.dram_tensor("hidq", [Q, D], fp, kind="ExternalInput").ap()
    wkd = nc.dram_tensor("wk", [D, D], f8, kind="ExternalInput").ap()
    wqd = nc.dram_tensor("wq", [D, D], f8, kind="ExternalInput").ap()
    wvcbd = nc.dram_tensor("wvcb", [D, WCB], f8, kind="ExternalInput").ap()
    wdd = nc.dram_tensor("wd", [D, D], f8, kind="ExternalInput").ap()
    mixbd = nc.dram_tensor("mixb", [P, NB * H * Q], bf,
                           kind="ExternalInput").ap()
    mqxd = nc.dram_tensor("mqx", [P, 2 * NPAIR * 2 * Q], f8,
                          kind="ExternalInput").ap()
    cnt8d = nc.dram_tensor("cnt8", [P, NKT * P], f8, kind="ExternalInput").ap()
    cntWd = nc.dram_tensor("cntW", [P, NKT], fp, kind="ExternalInput").ap()
    outd = nc.dram_tensor("out", [Q, D], fp, kind="ExternalOutput").ap()

    with tile.TileContext(nc) as tc:
        with (
            tc.tile_pool(name="singles", bufs=1) as singles,
            tc.tile_pool(name="acts", bufs=2) as acts,
            tc.tile_pool(name="mqp", bufs=2) as mqp,
            tc.tile_pool(name="wp", bufs=8) as wp,
            tc.tile_pool(name="ps", bufs=4, space="PSUM") as ps,
            tc.tile_pool(name="zp", bufs=2, space="PSUM") as zp,
        ):
            pools = {"singles": singles, "acts": acts, "mqp": mqp,
                     "wp": wp, "ps": ps, "zp": zp}
            # ---- constant / input loads (once) ----
            hT = singles.tile([P, NB, S], f8)
            hTr = hTd.rearrange("(o p) u -> p o u", p=P)
            for _o in range(NB):
                nc.sync.dma_start(hT[:, _o, :], hTr[:, _o, :])
            hTq = singles.tile([P, NB, Q], f8)
            nc.sync.dma_start(hTq[:], hTqd.rearrange("(o p) q -> p o q", p=P))
            wk_sb = singles.tile([P, NB, D], f8)
            wq_sb = singles.tile([P, NB, D], f8)
            wd_sb = singles.tile([P, NB, D], f8)
            for wdram, dest in ((wkd, wk_sb), (wqd, wq_sb), (wdd, wd_sb)):
                wr = wdram.rearrange("(o p) m -> p o m", p=P)
                for _o in range(NB):
                    nc.sync.dma_start(dest[:, _o, :], wr[:, _o, :])
            wvcb_sb = singles.tile([P, NB, WCB], f8)
            wvr = wvcbd.rearrange("(o p) m -> p o m", p=P)
            for _o in range(NB):
                nc.sync.dma_start(wvcb_sb[:, _o, :], wvr[:, _o, :])
            mixb_sb = singles.tile([P, NB, H, Q], bf)
            nc.sync.dma_start(mixb_sb[:], mixbd)
            mqx_sb = singles.tile([P, 2, NPAIR, 2, Q], f8)
            nc.sync.dma_start(mqx_sb[:], mqxd)
            cnt8_sb = singles.tile([P, NKT, P], f8)
            nc.sync.dma_start(cnt8_sb[:], cnt8d)
            cntW_sb = singles.tile([P, NKT], fp)
            nc.sync.dma_start(cntW_sb[:], cntWd)
            hidq_sb = singles.tile([Q, D], fp)
            nc.sync.dma_start(hidq_sb[:], hidqd)
            kTx_sb = singles.tile([P, 2, S], f8)
            nc.vector.memset(kTx_sb[:], 0.0)
            rsb_t = singles.tile([P, 1], fp)
            nc.vector.memset(rsb_t[:], 63.5 * math.log(2.0))

            io = {"hT": hT, "hTq": hTq, "wk": wk_sb, "wq": wq_sb,
                  "wvcb": wvcb_sb, "wd": wd_sb, "mixb": mixb_sb,
                  "mqx": mqx_sb, "cnt8": cnt8_sb, "cntW": cntW_sb,
                  "hidq_sb": hidq_sb, "kTx": kTx_sb, "rsb_t": rsb_t,
                  "out": outd}

            with nc.allow_low_precision(
                    reason="attention path tolerates fp8; residual+LN fp32"):
                for it in range(iters):
                    _emit(nc, tc, pools, io, it)

    nc.compile()
    return nc


def _get_nc(iters=1):
    key = ("nc", iters)
    if key not in _CACHE:
        _CACHE[key] = _build(iters)
    return _CACHE[key]


def _prepare_in_maps(hidden_states, fpos, tpos, Wq, Wk, Wcb, Wv, bv, mixing,
                     Wd, bd, ln_gamma, ln_beta):
    import ml_dtypes
    f8 = ml_dtypes.float8_e4m3
    bf16 = ml_dtypes.bfloat16
    hs = np.ascontiguousarray(np.asarray(hidden_states, dtype=np.float32))
    tidx = np.asarray(tpos).astype(np.int64) % S
    counts = np.bincount(tidx, minlength=S).astype(np.float32)
    wvcb = np.concatenate([np.asarray(Wv, np.float32),
                           np.asarray(Wcb, np.float32)], axis=1)
    # mixb[p, o, h, q] = 8*mix[h, o*128+p]/128, broadcast along q
    mixT = (MSCL / WSCL) * np.asarray(mixing, np.float32).T      # [D, H]
    mixb = np.broadcast_to(
        mixT.reshape(NB, P, H).transpose(1, 0, 2)[:, :, :, None],
        (P, NB, H, Q))
    # mqx[r, 0, pair, hh, q] = 1 if r == 2*pair+hh else 0; sub-block 1 is
    # all-zero (it pairs with the always-zero second kTx block under DR)
    mqx = np.zeros((P, 2, NPAIR, 2, Q), np.float32)
    for pair in range(NPAIR):
        for hh in range(2):
            mqx[2 * pair + hh, 0, pair, hh, :] = 1.0
    # cnt8[p, kt, c] = counts[kt*128+p]/32 ; cntW[p, kt] = counts[kt*128+p]/128
    cgrid = counts.reshape(NKT, P).T                              # [P, NKT]
    cnt8 = np.broadcast_to((cgrid / ZSCL)[:, :, None], (P, NKT, P))
    common = {
        "hT": np.ascontiguousarray(hs.T).astype(f8),
        "wk": (WSCL * np.asarray(Wk, np.float32)).astype(f8),
        "wq": (WSCL * np.asarray(Wq, np.float32)).astype(f8),
        "wvcb": (WSCL * wvcb).astype(f8),
        "wd": (WSCL * np.asarray(Wd, np.float32)).astype(f8),
        "mixb": np.ascontiguousarray(mixb.reshape(P, -1)).astype(bf16),
        "mqx": np.ascontiguousarray(mqx.reshape(P, -1)).astype(f8),
        "cnt8": np.ascontiguousarray(cnt8.reshape(P, -1)).astype(f8),
        "cntW": np.ascontiguousarray(cgrid / WSCL),
    }
    resid_bias = (np.asarray(bd, np.float32)
                  + np.asarray(bv, np.float32)
                  @ np.asarray(Wd, np.float32))
    in_maps = []
    for c in range(N_CORES):
        m = dict(common)
        rows = hs[Q * c: Q * (c + 1)]
        m["hTq"] = np.ascontiguousarray(rows.T).astype(f8)
        m["hidq"] = np.ascontiguousarray(rows + resid_bias[None, :])
        in_maps.append(m)
    return in_maps


def _run(inputs, trace=False, iters=1):
    from concourse import bass_utils
    nc = _get_nc(iters)
    in_maps = _prepare_in_maps(**inputs)
    res = bass_utils.run_bass_kernel_spmd(
        nc, in_maps, core_ids=list(range(N_CORES)), trace=trace)
    normedfull = np.concatenate(
        [np.asarray(res.results[c]["out"]) for c in range(N_CORES)], axis=0)
    gam = np.asarray(inputs["ln_gamma"], np.float32)
    bet = np.asarray(inputs["ln_beta"], np.float32)
    if not (np.all(gam == 1.0) and np.all(bet == 0.0)):
        normedfull = normedfull * gam[None, :] + bet[None, :]
    fidx = np.asarray(inputs["fpos"]).astype(np.int64) % S
    return np.ascontiguousarray(normedfull[fidx]), res


def kernel(**inputs) -> np.ndarray:
    out, _ = _run(inputs, trace=False)
    return out


# revision 45
# speedup vs baseline: 1.2149x; 1.0897x over previous
"""Trainium2 Bass kernel for CollaborativeAttention (row-sharded, fp8).

Math: with S=512 unique positions and F=T=2048 gathered via fpos/tpos (mod 512),
the whole block collapses to the unique-position problem:
    qf = hs @ Wq ; kf = hs @ Wk ; vf = hs @ Wv + bv ; cbf = hs @ Wcb       [512, *]
    per head h:  w[u, s] = exp(scale*((qf[u]*mix[h]) . kf[s] + cbf[s, h]))
    ctx[u, h*64:(h+1)*64] = (w @ (counts*vf)[:, hcols]) / (w @ counts)
    outfull = ctx @ Wd + bd ; resfull = hs + outfull ; LN  -> normedfull   [512, 1024]
    output  = normedfull[fpos % 512]                                       [2048, 1024]
counts[s] = multiplicity of s in (tpos % 512); the softmax over the 2048 keys
is the count-weighted softmax over the 512 unique keys, and the count weights
are folded multiplicatively into v (numerator) and the z-reduction lhsT
(denominator), so the exp has no per-key bias at all.

Distribution: collectives on this stack are far too slow (~ms), so the kernel
uses a zero-collective row shard: core c owns query rows 64c..64c+63. The k/v
projections (needed in full by every core) are replicated; everything else
(qT, scores, softmax, context, output dense, LayerNorm) is 1/8 per core. The
host concatenates the 8 disjoint row blocks and applies the fpos gather.

Layout/engine choices (cost-model driven — per-instruction fixed overheads on
Act/DVE dominate, so instruction count is minimized):
 - qT is produced directly by PE (lhsT=Wq-block, rhs=hTq), no transposes.
 - mq[d,h,q] = qT*8mix is two big broadcast tensor_tensor ops (one on DVE
   straight from PSUM, one on GpSimd from a small SBUF eviction) against a
   host-precomputed broadcast mixing table; not 128 small DVE ops.
 - the content bias enters the score PSUM via tiny indicator matmuls
   (kTx rows = 8*cb^T, static one-hot rhs), so Exp runs with bias=0 as one
   instruction per PSUM tile (8/iter instead of 64).
 - z and ctx are DoubleRow matmuls into full [128, 128] per-pair PSUM
   blocks (off-diagonal waste is free on PE), so the softmax normalize is
   one reciprocal + two diagonal multiplies with matching partition
   offsets on every operand (engines cannot shift partitions, and matmul
   outputs must start at partition 0).
 - 1/std uses the exponent-bits rsqrt estimate refined by one Newton step;
   the estimate's exp() shares the Exp activation table the scores already
   need, so the Act engine never reloads tables (Sqrt/Ln would cost two
   1.28us ACT_TABLE_LOADs per iteration).
 - the tail (output dense, residual, LayerNorm, store) of iteration i is
   emitted after iteration i+1's front: engines execute in program order,
   and this software pipelining fills the PE's wait on the softmax
   normalize with the next iteration's projection matmuls.

Precision: the residual path keeps the attention output at ~2% of the signal,
so the attention path runs in fp8 e4m3 with DoubleRow matmuls (fp32 PSUM);
the residual + LayerNorm stay fp32. Power-of-2 scale factors keep fp8
operands in the normal range: weights are pre-scaled x128 on the host
(compensated at PSUM eviction), mixing x8 (compensated in the exp scale), the
z-reduction lhsT holds counts/32 so the stored context is x32 (compensated by
a /4096 at the output-dense eviction). bd and bv@Wd are folded into the
host-prepared residual rows; ln_gamma/ln_beta are ones/zeros here (the host
re-applies them if they ever deviate). All weights are SBUF-resident.
"""

import math
import numpy as np

P = 128
S = 512
D = 1024
H = 16
DH = 64
Q = 64               # query rows per core
NB = D // P          # 8 contraction chunks
ND = NB // 2         # 4 DoubleRow double-chunks
NKT = S // P         # 4 key tiles
NPAIR = H // 2       # 8 head pairs
NPP = NPAIR // 2     # 4 pair-pairs (2 pairs share a psum tile)
WCB = D + H          # packed [Wv | Wcb] columns
N_CORES = 8
SCALE = 1.0 / math.sqrt(D / H)  # 0.125
LN_EPS = 1e-5

WSCL = 128.0         # host weight scale (power of 2, keeps fp8 normal-range)
MSCL = 8.0           # host mixing scale
ZSCL = 32.0          # context scale via counts/32 z-reduction
NPOOL = 5            # mq o-blocks computed on GpSimd (rest on DVE)

_CACHE = {}


def _emit(nc, tc, pools, io, it):
    """Emit one full compute iteration (everything after the constant loads)."""
    import concourse.mybir as mybir

    fp = mybir.dt.float32
    f8 = mybir.dt.float8e4
    bf = mybir.dt.bfloat16
    Alu = mybir.AluOpType
    Act = mybir.ActivationFunctionType
    DR = mybir.MatmulPerfMode.DoubleRow

    acts, wp, ps, zp = (pools[k] for k in ("acts", "wp", "ps", "zp"))
    hT = io["hT"]

    # ---- qT projection: qTp[dcol, o, q] = 128*q^T, PE-direct ----
    qTp = ps.tile([P, NB, Q], fp, tag="ps", name=f"qTp{it}")
    for o in range(NB):
        for oo in range(ND):
            nc.tensor.matmul(qTp[:, o, :],
                             lhsT=io["wq"][:, 2 * oo: 2 * oo + 2,
                                           P * o: P * (o + 1)],
                             rhs=io["hTq"][:, 2 * oo: 2 * oo + 2, :],
                             start=(o == 0 and oo == 0),
                             stop=(o == NB - 1 and oo == ND - 1),
                             perf_mode=DR, skip_group_check=True)
    # evict the whole qT to SBUF at once (128*qT, bf16) so the psum tile is
    # freed fast (GpSimd can't read PSUM anyway)
    NP0 = NB - NPOOL     # DVE handles blocks [0, NP0), Pool [NP0, NB)
    qTs = acts.tile([P, NB, Q], bf, tag="qTs", name=f"qTs{it}")
    nc.vector.tensor_scalar(out=qTs[:, NP0:NB, :], in0=qTp[:, NP0:NB, :],
                            scalar1=1.0, scalar2=None, op0=Alu.mult)
    nc.vector.tensor_scalar(out=qTs[:, 0:NP0, :], in0=qTp[:, 0:NP0, :],
                            scalar1=1.0, scalar2=None, op0=Alu.mult)
    # mq[d, o, h, q] = qT[d,o,q] * 8*mix[d,o,h]: mixb = 8*mix/128 broadcast
    mq = pools["mqp"].tile([P, NB, H, Q], f8, tag="mq", name=f"mq{it}")
    nc.gpsimd.tensor_tensor(
        out=mq[:, NP0:NB],
        in0=qTs[:, NP0:NB, :].unsqueeze(2).broadcast_to([P, NPOOL, H, Q]),
        in1=io["mixb"][:, NP0:NB], op=Alu.mult)
    nc.vector.tensor_tensor(
        out=mq[:, 0:NP0],
        in0=qTs[:, 0:NP0, :].unsqueeze(2).broadcast_to([P, NP0, H, Q]),
        in1=io["mixb"][:, 0:NP0], op=Alu.mult)

    # ---- k projection (full, replicated): kT_sb [d, keys], true scale ----
    kT = acts.tile([P, NB, S], f8, tag="kT", name=f"kT{it}")
    for j in range(NB):
        pk = ps.tile([P, S], fp, tag="ps", name=f"pk{it}_{j}")
        for oo in range(ND):
            nc.tensor.matmul(pk[:],
                             lhsT=io["wk"][:, 2 * oo: 2 * oo + 2,
                                           P * j: P * (j + 1)],
                             rhs=hT[:, 2 * oo: 2 * oo + 2, :],
                             start=(oo == 0), stop=(oo == ND - 1),
                             perf_mode=DR)
        if j < 7:
            nc.scalar.activation(kT[:, j, :], pk[:], Act.Copy, scale=1.0 / WSCL)
        else:
            nc.vector.tensor_scalar(out=kT[:, j, :], in0=pk[:],
                                    scalar1=1.0 / WSCL, scalar2=None,
                                    op0=Alu.mult)

    # ---- v projection (full, replicated), counts folded in at eviction ----
    v_sb = acts.tile([P, NKT, D], f8, tag="v", name=f"v{it}")
    for kt in range(NKT):
        pv = [ps.tile([P, S], fp, tag="ps", name=f"pv{it}_{kt}_{eh}")
              for eh in range(2)]
        for oo in range(ND):
            lhs = hT[:, 2 * oo: 2 * oo + 2, P * kt: P * (kt + 1)]
            for eh in range(2):
                nc.tensor.matmul(pv[eh][:], lhsT=lhs,
                                 rhs=io["wvcb"][:, 2 * oo: 2 * oo + 2,
                                                S * eh: S * (eh + 1)],
                                 start=(oo == 0), stop=(oo == ND - 1),
                                 perf_mode=DR)
        for eh in range(2):
            # v~ = counts * v  (scale is the per-partition counts/128 vector)
            if kt >= 2 and eh == 1:
                nc.vector.tensor_scalar(
                    out=v_sb[:, kt, S * eh: S * (eh + 1)], in0=pv[eh][:],
                    scalar1=io["cntW"][:, kt: kt + 1], scalar2=None,
                    op0=Alu.mult)
            else:
                nc.scalar.activation(v_sb[:, kt, S * eh: S * (eh + 1)],
                                     pv[eh][:], Act.Copy,
                                     scale=io["cntW"][:, kt: kt + 1])

    # ---- content bias, transposed: kTx[h, key] = 8*cb[key, h] ----
    pcbT = ps.tile([P, S], fp, tag="ps", name=f"pcbT{it}")
    for oo in range(ND):
        nc.tensor.matmul(pcbT[0:H, :],
                         lhsT=io["wvcb"][:, 2 * oo: 2 * oo + 2, D:WCB],
                         rhs=hT[:, 2 * oo: 2 * oo + 2, :],
                         start=(oo == 0), stop=(oo == ND - 1),
                         perf_mode=DR)
    # rows 16..127 and the whole second DR sub-block of kTx were zeroed once
    # in the preamble and stay zero
    nc.scalar.activation(io["kTx"][0:H, 0, :], pcbT[0:H, :], Act.Copy,
                         scale=MSCL / WSCL)

    # ---- scores -> exp -> z/ctx, two key-halves ----
    # psum sc[key, pa, ktl, (hh q)] = 8*(q.mix[h].k) + 8*cb[key, h]
    zps = zp.tile([P, NPAIR, P], fp, tag="zp", name=f"zps{it}")
    ctxps = zp.tile([P, NPAIR, P], fp, tag="zp", name=f"ctxps{it}")
    for kh in range(2):
        for pp in range(NPP):
            # sc[key, ktl, (pa hh q)]: one 256-wide moving operand covers the
            # tile's two head-pairs at once
            sc = ps.tile([P, 2, 2 * P], fp, tag="ps", name=f"sc{it}_{kh}_{pp}")
            # scores first (start=True clears the bank), content-bias DRs
            # last (they gate only the exp, not the score accumulation)
            for oo in range(ND):
                for ktl in range(2):
                    kt = 2 * kh + ktl
                    nc.tensor.matmul(
                        sc[:, ktl, :],
                        lhsT=kT[:, 2 * oo: 2 * oo + 2,
                                P * kt: P * (kt + 1)],
                        rhs=mq[:, 2 * oo: 2 * oo + 2,
                               4 * pp: 4 * pp + 4, :],
                        start=(oo == 0 and ktl == 0),
                        stop=False,
                        perf_mode=DR, skip_group_check=True)
            for ktl in range(2):
                kt = 2 * kh + ktl
                nc.tensor.matmul(sc[:, ktl, :],
                                 lhsT=io["kTx"][:, :, P * kt: P * (kt + 1)],
                                 rhs=io["mqx"][:, :, 2 * pp: 2 * pp + 2],
                                 start=False, stop=(ktl == 1),
                                 perf_mode=DR, skip_group_check=True)
            # exp: one instruction for the whole tile (no bias needed)
            wt = wp.tile([P, 2, 2 * P], f8, tag="w", name=f"w{it}_{kh}_{pp}")
            nc.scalar.activation(wt[:], sc[:], Act.Exp, scale=SCALE / MSCL)
            # z and ctx accumulate across both key-halves (DoubleRow over
            # the kh's two key tiles); full [128, *] outs, the diagonal
            # (matching head halves) is extracted in the normalize.
            # zps/ctxps each span TWO psum banks (pairs 0-3 / 4-7): the
            # first write into EACH bank must carry start=True
            first = (kh == 0 and pp % 2 == 0)
            last = (kh == 1 and pp == NPP - 1)
            nc.tensor.matmul(
                zps[:, 2 * pp: 2 * pp + 2, :],
                lhsT=io["cnt8"][:, 2 * kh: 2 * kh + 2, :],
                rhs=wt[:, :, :], start=first, stop=last,
                perf_mode=DR, skip_group_check=True)
            for pa in range(2):
                pair = 2 * pp + pa
                nc.tensor.matmul(
                    ctxps[:, pair, :],
                    lhsT=v_sb[:, 2 * kh: 2 * kh + 2,
                              P * pair: P * (pair + 1)],
                    rhs=wt[:, :, P * pa: P * (pa + 1)],
                    start=first and pa == 0, stop=last and pa == 1,
                    perf_mode=DR, skip_group_check=True)

    # ---- normalize: ctxT[vcol, pair, q] = 32*ctx[vcol, q]/z[head(vcol), q]
    # (engines may read only ONE psum operand per instruction, so z goes
    # through a reciprocal into SBUF first; zps holds the same z row on every
    # partition, so each partition half reads its own head half)
    rz = acts.tile([P, NPAIR, P], fp, tag="rz", name=f"rz{it}")
    nc.vector.reciprocal(rz[:], zps[:])
    ctxT = acts.tile([P, NPAIR, Q], f8, tag="ctxT", name=f"ctxT{it}")
    for hh in range(2):
        r0 = DH * hh
        nc.vector.tensor_tensor(ctxT[r0:r0 + DH, :, :],
                                ctxps[r0:r0 + DH, :, r0:r0 + DH],
                                rz[r0:r0 + DH, :, r0:r0 + DH], Alu.mult)

    # ---- output projection (psum = 32*ctx_n @ 128*Wd = 4096*out) ----
    po = [ps.tile([Q, S], fp, tag="ps", name=f"po{it}_{eh}")
          for eh in range(2)]
    for oo in range(ND):
        for eh in range(2):
            nc.tensor.matmul(po[eh][:],
                             lhsT=ctxT[:, 2 * oo: 2 * oo + 2, :],
                             rhs=io["wd"][:, 2 * oo: 2 * oo + 2,
                                          S * eh: S * (eh + 1)],
                             start=(oo == 0), stop=(oo == ND - 1),
                             perf_mode=DR)

    # ---- epilogue: residual (hid+bd+bv@Wd host-folded), LayerNorm ----
    r_sb = acts.tile([Q, D], fp, tag="r", name=f"r{it}")
    for eh in range(2):
        nc.vector.scalar_tensor_tensor(
            out=r_sb[:, S * eh: S * (eh + 1)], in0=po[eh][:],
            scalar=1.0 / (WSCL * ZSCL),
            in1=io["hidq_sb"][:, S * eh: S * (eh + 1)],
            op0=Alu.mult, op1=Alu.add)
    stats = acts.tile([Q, 2, 6], fp, tag="stats", name=f"stats{it}")
    nc.vector.bn_stats(stats[:, 0, :], r_sb[:, 0:S])
    nc.vector.bn_stats(stats[:, 1, :], r_sb[:, S:D])
    mv = acts.tile([Q, 2], fp, tag="mv", name=f"mv{it}")
    nc.vector.bn_aggr(mv[:], stats[:])
    # 1/std via exponent-bits rsqrt + one Newton step (avoids Ln, whose
    # activation table does not contain Exp -> 2x 1.28us table loads/iter):
    #   log2(v) ~ bits(v)/2^23 - 127 ; y0 = exp(-ln2/2 * log2(v))
    #   istd = y0*(1.5 - 0.5*v*y0^2)   (v >> eps here, so eps is dropped)
    y0 = acts.tile([Q, 1], fp, tag="y0", name=f"y0{it}")
    nc.scalar.activation(y0[:], mv[:, 1:2].bitcast(mybir.dt.int32), Act.Exp,
                         scale=-0.5 * math.log(2.0) / (1 << 23),
                         bias=io["rsb_t"][0:Q, :])
    t1 = acts.tile([Q, 1], fp, tag="t1", name=f"t1{it}")
    nc.vector.tensor_tensor(t1[:], y0[:], y0[:], Alu.mult)
    nc.vector.tensor_tensor(t1[:], t1[:], mv[:, 1:2], Alu.mult)
    nc.vector.tensor_scalar(out=t1[:], in0=t1[:], scalar1=-0.5, scalar2=1.5,
                            op0=Alu.mult, op1=Alu.add)
    std = acts.tile([Q, 1], fp, tag="std", name=f"std{it}")
    nc.vector.tensor_tensor(std[:], t1[:], y0[:], Alu.mult)
    nc.gpsimd.tensor_scalar(out=r_sb[:], in0=r_sb[:],
                            scalar1=mv[:, 0:1], scalar2=std[:],
                            op0=Alu.subtract, op1=Alu.mult)
    nc.sync.dma_start(io["out"][:], r_sb[:])


def _build(iters=1):
    import concourse.mybir as mybir
    import concourse.tile as tile
    from concourse import bacc

    fp = mybir.dt.float32
    f8 = mybir.dt.float8e4
    bf = mybir.dt.bfloat16

    nc = bacc.Bacc("TRN2", target_bir_lowering=False, debug=False,
                   num_devices=N_CORES)

    hTd = nc.dram_tensor("hT", [D, S], f8, kind="ExternalInput").ap()
    hTqd = nc.dram_tensor("hTq", [D, Q], f8, kind="ExternalInput").ap()
    hidqd = nc.dram_tensor("hidq", [Q, D], fp, kind="ExternalInput").ap()
    wkd = nc.dram_tensor("wk", [D, D], f8, kind="ExternalInput").ap()
    wqd = nc.dram_tensor("wq", [D, D], f8, kind="ExternalInput").ap()
    wvcbd = nc.dram_tensor("wvcb", [D, WCB], f8, kind="ExternalInput").ap()
    wdd = nc.dram_tensor("wd", [D, D], f8, kind="ExternalInput").ap()
    mixbd = nc.dram_tensor("mixb", [P, NB * H * Q], bf,
                           kind="ExternalInput").ap()
    mqxd = nc.dram_tensor("mqx", [P, 2 * NPAIR * 2 * Q], f8,
                          kind="ExternalInput").ap()
    cnt8d = nc.dram_tensor("cnt8", [P, NKT * P], f8, kind="ExternalInput").ap()
    cntWd = nc.dram_tensor("cntW", [P, NKT], fp, kind="ExternalInput").ap()
    outd = nc.dram_tensor("out", [Q, D], fp, kind="ExternalOutput").ap()

    with tile.TileContext(nc) as tc:
        with (
            tc.tile_pool(name="singles", bufs=1) as singles,
            tc.tile_pool(name="acts", bufs=3) as acts,
            tc.tile_pool(name="mqp", bufs=3) as mqp,
            tc.tile_pool(name="wp", bufs=12) as wp,
            tc.tile_pool(name="ps", bufs=4, space="PSUM") as ps,
            tc.tile_pool(name="zp", bufs=2, space="PSUM") as zp,
        ):
            pools = {"singles": singles, "acts": acts, "mqp": mqp,
                     "wp": wp, "ps": ps, "zp": zp}
            # ---- constant / input loads (once) ----
            hT = singles.tile([P, NB, S], f8)
            hTr = hTd.rearrange("(o p) u -> p o u", p=P)
            for _o in range(NB):
                nc.sync.dma_start(hT[:, _o, :], hTr[:, _o, :])
            hTq = singles.tile([P, NB, Q], f8)
            nc.sync.dma_start(hTq[:], hTqd.rearrange("(o p) q -> p o q", p=P))
            wk_sb = singles.tile([P, NB, D], f8)
            wq_sb = singles.tile([P, NB, D], f8)
            wd_sb = singles.tile([P, NB, D], f8)
            for wdram, dest in ((wkd, wk_sb), (wqd, wq_sb), (wdd, wd_sb)):
                wr = wdram.rearrange("(o p) m -> p o m", p=P)
                for _o in range(NB):
                    nc.sync.dma_start(dest[:, _o, :], wr[:, _o, :])
            wvcb_sb = singles.tile([P, NB, WCB], f8)
            wvr = wvcbd.rearrange("(o p) m -> p o m", p=P)
            for _o in range(NB):
                nc.sync.dma_start(wvcb_sb[:, _o, :], wvr[:, _o, :])
            mixb_sb = singles.tile([P, NB, H, Q], bf)
            nc.sync.dma_start(mixb_sb[:], mixbd)
            mqx_sb = singles.tile([P, 2, NPAIR, 2, Q], f8)
            nc.sync.dma_start(mqx_sb[:], mqxd)
            cnt8_sb = singles.tile([P, NKT, P], f8)
            nc.sync.dma_start(cnt8_sb[:], cnt8d)
            cntW_sb = singles.tile([P, NKT], fp)
            nc.sync.dma_start(cntW_sb[:], cntWd)
            hidq_sb = singles.tile([Q, D], fp)
            nc.sync.dma_start(hidq_sb[:], hidqd)
            kTx_sb = singles.tile([P, 2, S], f8)
            nc.vector.memset(kTx_sb[:], 0.0)
            rsb_t = singles.tile([P, 1], fp)
            nc.vector.memset(rsb_t[:], 63.5 * math.log(2.0))

            io = {"hT": hT, "hTq": hTq, "wk": wk_sb, "wq": wq_sb,
                  "wvcb": wvcb_sb, "wd": wd_sb, "mixb": mixb_sb,
                  "mqx": mqx_sb, "cnt8": cnt8_sb, "cntW": cntW_sb,
                  "hidq_sb": hidq_sb, "kTx": kTx_sb, "rsb_t": rsb_t,
                  "out": outd}

            with nc.allow_low_precision(
                    reason="attention path tolerates fp8; residual+LN fp32"):
                for it in range(iters):
                    _emit(nc, tc, pools, io, it)

    nc.compile()
    return nc


def _get_nc(iters=1):
    key = ("nc", iters)
    if key not in _CACHE:
        _CACHE[key] = _build(iters)
    return _CACHE[key]


def _prepare_in_maps(hidden_states, fpos, tpos, Wq, Wk, Wcb, Wv, bv, mixing,
                     Wd, bd, ln_gamma, ln_beta):
    import ml_dtypes
    f8 = ml_dtypes.float8_e4m3
    bf16 = ml_dtypes.bfloat16
    hs = np.ascontiguousarray(np.asarray(hidden_states, dtype=np.float32))
    tidx = np.asarray(tpos).astype(np.int64) % S
    counts = np.bincount(tidx, minlength=S).astype(np.float32)
    wvcb = np.concatenate([np.asarray(Wv, np.float32),
                           np.asarray(Wcb, np.float32)], axis=1)
    # mixb[p, o, h, q] = 8*mix[h, o*128+p]/128, broadcast along q
    mixT = (MSCL / WSCL) * np.asarray(mixing, np.float32).T      # [D, H]
    mixb = np.broadcast_to(
        mixT.reshape(NB, P, H).transpose(1, 0, 2)[:, :, :, None],
        (P, NB, H, Q))
    # mqx[r, 0, pair, hh, q] = 1 if r == 2*pair+hh else 0; sub-block 1 is
    # all-zero (it pairs with the always-zero second kTx block under DR)
    mqx = np.zeros((P, 2, NPAIR, 2, Q), np.float32)
    for pair in range(NPAIR):
        for hh in range(2):
            mqx[2 * pair + hh, 0, pair, hh, :] = 1.0
    # cnt8[p, kt, c] = counts[kt*128+p]/32 ; cntW[p, kt] = counts[kt*128+p]/128
    cgrid = counts.reshape(NKT, P).T                              # [P, NKT]
    cnt8 = np.broadcast_to((cgrid / ZSCL)[:, :, None], (P, NKT, P))
    common = {
        "hT": np.ascontiguousarray(hs.T).astype(f8),
        "wk": (WSCL * np.asarray(Wk, np.float32)).astype(f8),
        "wq": (WSCL * np.asarray(Wq, np.float32)).astype(f8),
        "wvcb": (WSCL * wvcb).astype(f8),
        "wd": (WSCL * np.asarray(Wd, np.float32)).astype(f8),
        "mixb": np.ascontiguousarray(mixb.reshape(P, -1)).astype(bf16),
        "mqx": np.ascontiguousarray(mqx.reshape(P, -1)).astype(f8),
        "cnt8": np.ascontiguousarray(cnt8.reshape(P, -1)).astype(f8),
        "cntW": np.ascontiguousarray(cgrid / WSCL),
    }
    resid_bias = (np.asarray(bd, np.float32)
                  + np.asarray(bv, np.float32)
                  @ np.asarray(Wd, np.float32))
    in_maps = []
    for c in range(N_CORES):
        m = dict(common)
        rows = hs[Q * c: Q * (c + 1)]
        m["hTq"] = np.ascontiguousarray(rows.T).astype(f8)
        m["hidq"] = np.ascontiguousarray(rows + resid_bias[None, :])
        in_maps.append(m)
    return in_maps


def _run(inputs, trace=False, iters=1):
    from concourse import bass_utils
    nc = _get_nc(iters)
    in_maps = _prepare_in_maps(**inputs)
    res = bass_utils.run_bass_kernel_spmd(
        nc, in_maps, core_ids=list(range(N_CORES)), trace=trace)
    normedfull = np.concatenate(
        [np.asarray(res.results[c]["out"]) for c in range(N_CORES)], axis=0)
    gam = np.asarray(inputs["ln_gamma"], np.float32)
    bet = np.asarray(inputs["ln_beta"], np.float32)
    if not (np.all(gam == 1.0) and np.all(bet == 0.0)):
        normedfull = normedfull * gam[None, :] + bet[None, :]
    fidx = np.asarray(inputs["fpos"]).astype(np.int64) % S
    return np.ascontiguousarray(normedfull[fidx]), res


def kernel(**inputs) -> np.ndarray:
    out, _ = _run(inputs, trace=False)
    return out
